# revision 94
# baseline (speedup 1.0000x reference)
"""DSS Mamba (bidirectional selective scan) Trainium2 kernel.

Sharding: 8 cores = 2 directions x 2 batch x 2 halves of d_inner.
Each core:
  - computes in_proj (x rows for its whole direction, z rows for its half),
  - x_proj -> (dt, B, C), dt_proj -> softplus -> delta,
  - selective scan over its 256 channels (d on partitions, L on free dim,
    hardware tensor_tensor_scan along the free dim, 16 states sequentially),
  - gate + partial out_proj (its 256 rows of the 1024-row contraction).
Host flips the sequence for the backward direction and sums the 4 partial
out_proj contributions per batch element.

Engine split per (state n, chunk): ACT computes dA=exp(A_n*delta); the DVE
runs every tensor_tensor_scan (fp32 state, bf16 out; the scan opcode is
DVE-only -- walrus rejects it on Pool) plus the bf16-2x broadcast multiplies
dBu=du*B_n and t=h*C_n for even states; GPSIMD takes the multiplies for odd
states to offload the DVE; the TensorEngine accumulates sum_n t_n (plus a
diag(D) matmul for the D*u term) into PSUM.  All matmuls and working tiles
are bf16 (fp32 PSUM accumulation); the two 128-channel halves share
[128, 2*LC] tiles so wide elementwise ops cover both.

The kernel is software-pipelined over two 1024-column L-chunks: prestage of
chunk lc+1 is emitted in four pieces inside chunk lc's scan loop (in_proj
fc0 at state PRE_A, fc1 at PRE_A+1, dt_proj+softplus-exp at PRE_B1,
softplus-ln+du at PRE_B) so the ACT dA stream is interrupted only briefly
(the da ring covers the gap), PE's in-order queue gets the next chunk's
matmuls before the scan-paced yps accumulations can delay them, and the
first EARLY states of chunk lc+1 are emitted before poststage(lc) so
DVE/GPSIMD never drain at the chunk boundary.  An activation-table patch
keeps Exp/Ln in one table so table reloads drop from ~15 to ~7 total.
"""

import numpy as np
from contextlib import ExitStack

import concourse.bacc as bacc
import concourse.tile as tile
from concourse import mybir
from concourse.bass_utils import run_bass_kernel_spmd

F32 = mybir.dt.float32
BF16 = mybir.dt.bfloat16
AF = mybir.ActivationFunctionType
OP = mybir.AluOpType

D_MODEL = 256
D_INNER = 512
N_STATE = 16
DT_RANK = 16
import os
L = 2048
LC = int(os.environ.get("K_LC", "1024"))   # pipeline chunk of L
N_LC = L // LC
FC = 512           # matmul free-dim chunk
N_FCC = LC // FC   # matmul chunks per pipeline chunk
DAB = int(os.environ.get("K_DAB", "10"))      # da tile ring size (ACT run-ahead)
GPMAX = int(os.environ.get("K_GPMAX", "15"))  # odd states below this get GP multiplies

_CACHE = {}


def _patch_act_tables():
    """Make the act-table chooser resolve Exp to the table that also holds Ln.

    softplus needs Exp then Ln back to back and the dA exponentials follow
    immediately; if Exp resolves to a table without Ln every exp<->ln boundary
    reloads the activation table (~1.3us each).  Removing Exp from the other
    sets (set order, and hence act_func_set ids, unchanged) forces the chooser
    onto natural_log_exp_and_others.
    """
    import concourse.hw_specs as hw_specs
    import concourse.bacc as bacc_mod
    if getattr(hw_specs.get_activation_tables, "_exp_patched", False):
        return
    orig = hw_specs.get_activation_tables

    def patched(arch):
        tabs = dict(orig(arch))
        exp = mybir.ActivationFunctionType.Exp
        ln = mybir.ActivationFunctionType.Ln
        out = {}
        for name, s in tabs.items():
            if name != "natural_log_exp_and_others":
                s = s - {exp, ln}
            out[name] = s
        return out

    patched._exp_patched = True
    hw_specs.get_activation_tables = patched
    bacc_mod.get_activation_tables = patched


def _build():
    if "nc" in _CACHE:
        return _CACHE["nc"]

    _patch_act_tables()
    nc = bacc.Bacc("TRN2", target_bir_lowering=False, debug=False)

    def din(name, shape, dtype=F32):
        return nc.dram_tensor(name, shape, dtype, kind="ExternalInput").ap()

    hsT = din("hsT", [2, 128, L], BF16)
    w_in_x = din("w_in_x", [2, 128, 512], BF16)
    w_in_z = din("w_in_z", [2, 128, 256], BF16)
    w_x = din("w_x", [4, 128, 64], BF16)
    w_dt = din("w_dt", [16, 256], BF16)
    bdt = din("bdt", [2, 128, 1])
    a_sc = din("a_sc", [2, 128, 16])         # A[d, n] per my d rows
    ddiag = din("ddiag", [2, 128, 128], BF16)  # diag(D) per m half
    w_out = din("w_out", [2, 128, 256], BF16)
    ident = nc.dram_tensor("ident", [128, 128], BF16, kind="ExternalInput").ap()
    out_ap = nc.dram_tensor("out", [2, 128, L], BF16, kind="ExternalOutput").ap()
    # B/C rows staged to DRAM (bf16) for partition-broadcast DMA reads
    bc_dram = nc.dram_tensor("bc_scratch", [32, L], BF16).ap()

    with tile.TileContext(nc) as tc, ExitStack() as ctx:
        const = ctx.enter_context(tc.tile_pool(name="const", bufs=1))
        big = ctx.enter_context(tc.tile_pool(name="big", bufs=2))
        work = ctx.enter_context(tc.tile_pool(name="work", bufs=int(os.environ.get("K_WORKB", "2"))))
        psum = ctx.enter_context(tc.tile_pool(name="psum", bufs=3, space="PSUM"))
        psum48 = ctx.enter_context(tc.tile_pool(name="psum48", bufs=1, space="PSUM"))
        psumy = ctx.enter_context(tc.tile_pool(name="psumy", bufs=int(os.environ.get("K_PSUMYB", "1")), space="PSUM"))

        # ---- load weights: first-needed ones via ACT, the rest via the Pool
        # queue (idle during pipeline fill), ordered by first use ----
        def load_const(ap, shape, tag, dtype=F32, eng=None):
            t = const.tile(shape, dtype, tag=tag, name=tag)
            (eng or nc.gpsimd).dma_start(out=t[:], in_=ap)
            return t

        # winx via SP, ahead of the hsk loads: the ACT sequencer spends its
        # first ~1.3us on the initial act-table load, which would delay these
        w_in_x_sb = [load_const(w_in_x[k], [128, 512], f"winx{k}", BF16, nc.sync)
                     for k in range(2)]
        w_x_sb = [load_const(w_x[k], [128, 64], f"wx{k}", BF16) for k in range(4)]
        w_dt_sb = load_const(w_dt, [16, 256], "wdt", BF16)
        bdt_sb = [load_const(bdt[m], [128, 1], f"bdt{m}") for m in range(2)]
        # winz via ACT: z-silus sit mid-ACT-stream, so a late winz would
        # head-block the whole silu block
        w_in_z_sb = [load_const(w_in_z[k], [128, 256], f"winz{k}", BF16, nc.scalar)
                     for k in range(2)]
        a_sc_sb = [load_const(a_sc[m], [128, 16], f"asc{m}") for m in range(2)]
        # ident/ddiag/wout are loaded after prestage(0) is emitted (see below)
        # so chunk 0's bc_dram staging write isn't queued behind them on Pool
        late_weights = {}

        def load_late_weights():
            late_weights["ident"] = load_const(ident, [128, 128], "ident", BF16)
            late_weights["ddiag"] = [load_const(ddiag[m], [128, 128], f"ddiag{m}", BF16)
                                     for m in range(2)]
            late_weights["wout"] = [load_const(w_out[k], [128, 256], f"wout{k}", BF16)
                                    for k in range(2)]

        hlast = [[const.tile([128, 1], F32, tag=f"hl{m}_{n}", name=f"hl{m}_{n}")
                  for n in range(N_STATE)] for m in range(2)]

        # dummy silu at priority ~0: pulls the compiler-inserted silu act-table
        # load to the very start of the ACT queue (otherwise it schedules at
        # ~5.8us, right before the first real silu, lengthening the fill)
        warm = const.tile([128, 1], F32, tag="warm", name="warm")
        nc.vector.memset(warm[:], 0.0)
        nc.scalar.activation(warm[:], warm[:], AF.Silu)

        # m-merged layout: the two 128-channel halves (m=0,1) of this core's
        # scan live side by side in [128, 2*LC] tiles; column m*LC+t holds
        # channel-half m, chunk-local timestep t.  Elementwise ops over both
        # halves are then single wide DVE instructions.
        L2 = 2 * LC

        def mcol(m, fc):
            return slice((m % 2) * LC + fc * FC, (m % 2) * LC + (fc + 1) * FC)

        z_pend = {}

        def prestage_a(lc, fcs=None, with_z=True):
            """Projections part A: in_proj+silu, x_proj, B/C staging."""
            base = lc * LC
            u01 = big.tile([128, L2], BF16, tag="u01", name="u01")
            u23 = big.tile([128, L2], BF16, tag="u23", name="u23")
            z2 = big.tile([128, L2], BF16, tag="z2", name="z2")
            xdbl = big.tile([64, LC], BF16, tag="xdbl", name="xdbl")
            st = (u01, u23, z2, xdbl)
            prestage_a_fc(lc, st, range(N_FCC) if fcs is None else fcs, with_z)
            return st

        def prestage_a_fc(lc, st, fcs, with_z=True):
            base = lc * LC
            u01, u23, z2, xdbl = st
            uof = lambda m: (u01 if m < 2 else u23)
            for fc in fcs:
                fs = slice(fc * FC, (fc + 1) * FC)       # within-chunk
                gs = slice(base + fc * FC, base + (fc + 1) * FC)  # global
                hsk = []
                for k in range(2):
                    t = work.tile([128, FC], BF16, tag=f"hsk{k}", name=f"hsk{k}")
                    nc.sync.dma_start(out=t[:], in_=hsT[k][:, gs])
                    hsk.append(t)
                for m in range(4):
                    ps = psum.tile([128, FC], F32, tag="mm", name="mm")
                    for k in range(2):
                        nc.tensor.matmul(ps[:], lhsT=w_in_x_sb[k][:, m * 128:(m + 1) * 128],
                                         rhs=hsk[k][:], start=(k == 0), stop=(k == 1))
                    nc.scalar.activation(uof(m)[:, mcol(m, fc)], ps[:], AF.Silu)
                if with_z:
                    emit_z(lc, st, fc, hsk)
                else:
                    # chunk 0: defer the z path off the fill-critical ACT
                    # stream; emitted later merged into the next silu block
                    z_pend.setdefault(lc, []).append((fc, hsk))
                ps48 = psum48.tile([64, FC], F32, tag="mm48", name="mm48")
                for k in range(4):
                    nc.tensor.matmul(ps48[:], lhsT=w_x_sb[k][:],
                                     rhs=uof(k)[:, mcol(k, fc)],
                                     start=(k == 0), stop=(k == 3))
                nc.vector.tensor_copy(xdbl[:, fs], ps48[:])
                if fc == N_FCC - 1:
                    # gpsimd-issued: costs ~1us of Pool engine time, but SP
                    # placement head-blocks the bb/cb issue stream at runtime
                    # (in-order queue) and ACT placement stalls the dA stream
                    nc.gpsimd.dma_start(out=bc_dram[:, base:base + LC],
                                        in_=xdbl[32:64, :])

        def emit_z(lc, st, fc, hsk):
            u01, u23, z2, xdbl = st
            for m in range(2):
                ps = psum.tile([128, FC], F32, tag="mm", name="mm")
                for k in range(2):
                    nc.tensor.matmul(ps[:], lhsT=w_in_z_sb[k][:, m * 128:(m + 1) * 128],
                                     rhs=hsk[k][:], start=(k == 0), stop=(k == 1))
                nc.scalar.activation(z2[:, mcol(m, fc)], ps[:], AF.Silu)

        def emit_z_pending(lc, st):
            for fc, hsk in z_pend.pop(lc, []):
                emit_z(lc, st, fc, hsk)

        def prestage_b1(lc, st):
            """Projections part B1: dt_proj matmuls + softplus exp.

            The dt matmuls must land early in the PE queue (behind scan-paced
            yps accumulations they would execute ~20us late and starve the
            next chunk's multiplies); the sp exps share the exp/ln table with
            dA, so this detour costs no table reload and frees the PSUM ring.
            """
            u01, u23, z2, xdbl = st
            sps = []
            for m in range(2):
                for fc in range(N_FCC):
                    fs = slice(fc * FC, (fc + 1) * FC)
                    ps = psum.tile([128, FC], F32, tag="mm", name="mm")
                    nc.tensor.matmul(ps[:], lhsT=w_dt_sb[:, m * 128:(m + 1) * 128],
                                     rhs=xdbl[0:16, fs], start=True, stop=True)
                    # softplus(x) = ln(exp(x) + 1); x = raw + bdt stays < ~3 here
                    sp = work.tile([128, FC], F32, tag="sp", name="sp", bufs=5)
                    nc.scalar.activation(sp[:], ps[:], AF.Exp, bias=bdt_sb[m][:])
                    sps.append((m, fc, sp))
            return sps

        def prestage_b2(lc, st, sps):
            """Projections part B2: softplus ln + du."""
            u01, u23, z2, xdbl = st
            # bf16 delta: the 0.4% rounding enters dA as exp(-n*delta*(1+eps));
            # the accumulated scan log-error is ~0.004*sqrt(n*delta*T_mem) with
            # T_mem ~ 1/(n*delta), i.e. bounded by ~0.4% regardless of delta.
            delta2 = big.tile([128, L2], BF16, tag="delta2", name="delta2")
            du2 = big.tile([128, L2], BF16, tag="du2", name="du2")
            for m in range(2):
                for mm, fc, sp in sps:
                    if mm == m:
                        nc.scalar.activation(delta2[:, mcol(m, fc)], sp[:], AF.Ln,
                                             bias=1.0)
                ms = slice(m * LC, (m + 1) * LC)
                nc.vector.tensor_tensor(du2[:, ms], delta2[:, ms], u01[:, ms],
                                        OP.mult)
            return u01, u23, z2, delta2, du2

        def prestage(lc):
            st = prestage_a(lc)
            return prestage_b2(lc, st, prestage_b1(lc, st))

        def make_yps(lc):
            return psumy.tile([128, L2], F32, tag="yps2", name="yps2")

        def scan_pre(lc, n, du2):
            """B/C broadcast DMAs + the dbu multiplies for state n.

            Emitted ahead of the scan (lookahead) so the final states' scans
            are not serialized behind their own dbu at the chunk tail, and
            GPSIMD works on dbu(n+2) while t(n) still waits for its scan.
            The scan itself is DVE-only (TensorScalarPtr is illegal on Pool);
            GPSIMD instead absorbs the B/C broadcast multiplies for a subset
            of states (GP TensorTensor runs at ~3.6x the DVE bf16 cost, so
            roughly 7 of 16 states' multiplies balance the two queues).
            """
            gl = slice(lc * LC, (lc + 1) * LC)
            bb = work.tile([128, LC], BF16, tag="bb", name="bb", bufs=6)
            nc.sync.dma_start(out=bb[:],
                              in_=bc_dram[n:n + 1, gl].to_broadcast([128, LC]))
            cb = work.tile([128, LC], BF16, tag="cb", name="cb", bufs=6)
            nc.sync.dma_start(out=cb[:],
                              in_=bc_dram[16 + n:17 + n, gl].to_broadcast([128, LC]))
            mult_eng = nc.gpsimd if (n % 2 == 1 and n < GPMAX) else nc.vector
            dbus = []
            for m in range(2):
                ms = slice(m * LC, (m + 1) * LC)
                dbu = work.tile([128, LC], BF16, tag="dbu", name="dbu", bufs=10)
                mult_eng.tensor_tensor(dbu[:], du2[:, ms], bb[:], OP.mult)
                dbus.append(dbu)
            return cb, dbus, mult_eng

        def scan_n(lc, n, yps, delta2, preh, zero_init, save_h, first=None):
            """Scan state n (both m halves) over chunk lc."""
            cb, dbus, mult_eng = preh
            for m in range(2):
                ms = slice(m * LC, (m + 1) * LC)
                da = work.tile([128, LC], F32, tag="da", name="da", bufs=DAB)
                nc.scalar.activation(da[:], delta2[:, ms], AF.Exp,
                                     scale=a_sc_sb[m][:, n:n + 1])
                dbu = dbus[m]
                h = work.tile([128, LC], BF16, tag=f"h{m}", name=f"h{m}", bufs=5)
                init = 0.0 if zero_init else hlast[m][n][:]
                nc.vector.tensor_tensor_scan(h[:], da[:], dbu[:], init,
                                             OP.mult, OP.add)
                if save_h:
                    # keep the saves off the Pool queue entirely: a pending
                    # copy at its head would stall the next chunk's multiplies
                    if n >= 13:
                        nc.vector.tensor_copy(hlast[m][n][:], h[:, LC - 1:LC])
                    else:
                        nc.scalar.copy(hlast[m][n][:], h[:, LC - 1:LC])
                t = work.tile([128, LC], BF16, tag="t", name="t", bufs=6)
                mult_eng.tensor_tensor(t[:], h[:], cb[:], OP.mult)
                for q in range(LC // 512):
                    nc.tensor.matmul(yps[:, m * LC + q * 512:m * LC + (q + 1) * 512],
                                     lhsT=late_weights["ident"][:], rhs=t[:, q * 512:(q + 1) * 512],
                                     start=(first if first is not None else (n == 0)), stop=False,
                                     skip_group_check=True)

        def poststage(lc, u01, u23, z2, delta2, du2, yps):
            """Gate + out_proj partial for chunk lc, at FC granularity."""
            base = lc * LC
            y = big.tile([128, L2], BF16, tag="y", name="y")
            # per 512-column region: close the PSUM accumulation group with
            # the D*u diag matmul, then gate just that region -- so out_proj
            # for fc0 starts without waiting for the whole chunk's gate
            for fc in range(N_FCC):
                for m in range(2):
                    cs = slice(m * LC + fc * FC, m * LC + (fc + 1) * FC)
                    nc.tensor.matmul(yps[:, cs], lhsT=late_weights["ddiag"][m][:],
                                     rhs=u01[:, cs], start=False, stop=True,
                                     skip_group_check=True)
                for m in range(2):
                    cs = slice(m * LC + fc * FC, m * LC + (fc + 1) * FC)
                    nc.vector.tensor_tensor(y[:, cs], yps[:, cs], z2[:, cs],
                                            OP.mult)
            for fc in range(N_FCC):
                fs = slice(fc * FC, (fc + 1) * FC)
                gs = slice(base + fc * FC, base + (fc + 1) * FC)
                for oi in range(2):
                    ps = psum.tile([128, FC], F32, tag="mm", name="mm")
                    for k in range(2):
                        nc.tensor.matmul(ps[:], lhsT=late_weights["wout"][k][:, oi * 128:(oi + 1) * 128],
                                         rhs=y[:, k * LC + fc * FC:k * LC + (fc + 1) * FC],
                                         start=(k == 0), stop=(k == 1))
                    # staging copy + out DMA both on ACT: keeps the SP queue
                    # free for bb/cb broadcasts (no head-of-line block on the
                    # gate chain).
                    ob = work.tile([128, FC], BF16, tag="ob", name="ob", bufs=4)
                    nc.scalar.copy(ob[:], ps[:])
                    nc.scalar.dma_start(out=out_ap[oi][:, gs], in_=ob[:])

        # ---- software pipeline over L-chunks ----
        # prestage(lc+1) is emitted right after state n=2 of chunk lc's scan:
        # its PE matmuls slot into PE idle time immediately, its ACT block
        # (silu -> softplus) interrupts the dA stream once (da buffers cover
        # the gap), and delta/du for lc+1 are ready long before the scans
        # need them -- so the chunk transition has no engine drain.
        PRE_A = int(os.environ.get("K_PREA", "6"))
        PRE_B1 = int(os.environ.get("K_PREB1", "7"))
        PRE_B = int(os.environ.get("K_PREB", "9"))
        EARLY = int(os.environ.get("K_EARLY", "2"))
        LOOKAHEAD = int(os.environ.get("K_LOOKAHEAD", "2"))
        pend = {}

        def ensure_pre(lc_, n_, du2_):
            if (lc_, n_) not in pend:
                pend[(lc_, n_)] = scan_pre(lc_, n_, du2_)

        pre = prestage(0)
        load_late_weights()
        yps = make_yps(0)
        n0 = 0
        for lc in range(N_LC):
            nxt = st = sps = None
            order = list(range(n0, N_STATE))
            for i, n in enumerate(order):
                for la in range(n, min(n + 1 + LOOKAHEAD, N_STATE)):
                    ensure_pre(lc, la, pre[4])
                scan_n(lc, n, yps, pre[3], pend.pop((lc, n)),
                       zero_init=(lc == 0),
                       save_h=(lc + 1 < N_LC), first=(i == 0 and n0 == 0))
                if lc + 1 < N_LC:
                    if n == PRE_A:
                        emit_z_pending(lc, (pre[0], pre[1], pre[2], None))
                        st = prestage_a(lc + 1, fcs=[0])
                    elif n == PRE_A + 1 and PRE_A + 1 < PRE_B1:
                        prestage_a_fc(lc + 1, st, [1])
                    elif n == PRE_B1:
                        if PRE_A + 1 >= PRE_B1:
                            prestage_a_fc(lc + 1, st, [1])
                        sps = prestage_b1(lc + 1, st)
                    elif n == PRE_B:
                        nxt = prestage_b2(lc + 1, st, sps)
            # emit the first few next-chunk states before poststage: their
            # DVE/Pool multiplies start during the current chunk's tail, and
            # only the PE accumulation waits for the gate's PSUM read
            yps_done, pre_done = yps, pre
            n0 = 0
            if nxt is not None:
                yps = make_yps(lc + 1)
                for n in range(EARLY):
                    for la in range(n, min(n + 1 + LOOKAHEAD, N_STATE)):
                        ensure_pre(lc + 1, la, nxt[4])
                    scan_n(lc + 1, n, yps, nxt[3], pend.pop((lc + 1, n)),
                           zero_init=False, save_h=(lc + 2 < N_LC),
                           first=(n == 0))
                n0 = EARLY
            poststage(lc, *pre_done, yps_done)
            pre = nxt

    nc.compile()
    _CACHE["nc"] = nc
    return nc


def _in_maps(inputs):
    import ml_dtypes
    f = lambda a: np.ascontiguousarray(np.asarray(a), dtype=np.float32)
    g = lambda a: np.ascontiguousarray(np.asarray(a, dtype=np.float32), dtype=ml_dtypes.bfloat16)
    hs = f(inputs["hidden_states"])          # [2, L, 256]
    W_in = f(inputs["W_in"])                 # [2048, 256]
    W_out = f(inputs["W_out"])               # [256, 1024]
    ident = np.eye(128, dtype=ml_dtypes.bfloat16)
    maps = []
    for branch in range(2):
        sfx = "f" if branch == 0 else "b"
        Wx0 = f(inputs[f"Wx_{sfx}"])         # [48, 512]
        Wx = np.zeros((64, 512), np.float32)  # dt rows 0:16, B 32:48, C 48:64
        Wx[0:16] = Wx0[0:16]
        Wx[32:48] = Wx0[16:32]
        Wx[48:64] = Wx0[32:48]
        Wdt = f(inputs[f"Wdt_{sfx}"])        # [512, 16]
        bdt = f(inputs[f"bdt_{sfx}"])        # [512]
        A = -np.exp(f(inputs[f"A_log_{sfx}"]))   # [512, 16]
        D = f(inputs[f"D_{sfx}"])            # [512]
        xrows = W_in[branch * 1024: branch * 1024 + 512]
        zrows = W_in[branch * 1024 + 512: branch * 1024 + 1024]
        for b in range(2):
            hsT = hs[b].T                    # [256, L]
            if branch == 1:
                hsT = hsT[:, ::-1]
            for half in range(2):
                mine = np.arange(256 * half, 256 * half + 256)
                perm = np.r_[mine, np.arange(256 * (1 - half), 256 * (1 - half) + 256)]
                m = {
                    "hsT": g(hsT).reshape(2, 128, L),
                    "w_in_x": g(xrows[perm].T).reshape(2, 128, 512),
                    "w_in_z": g(zrows[mine].T).reshape(2, 128, 256),
                    "w_x": g(Wx[:, perm].T).reshape(4, 128, 64),
                    "w_dt": g(Wdt[mine].T),
                    "bdt": f(bdt[mine]).reshape(2, 128, 1),
                    "a_sc": f(A[mine]).reshape(2, 128, 16),
                    "ddiag": np.stack([np.diag(g(D[mine][128 * i:128 * i + 128]))
                                       for i in range(2)]).astype(ident.dtype),
                    "w_out": g(W_out[:, branch * 512 + 256 * half:
                                     branch * 512 + 256 * half + 256].T).reshape(2, 128, 256),
                    "ident": ident,
                }
                maps.append(m)
    # maps order: branch-major, then b, then half -> core = (branch*2+b)*2+half
    return maps


def _run(inputs, trace=False):
    nc = _build()
    maps = _in_maps(inputs)
    res = run_bass_kernel_spmd(nc, maps, core_ids=list(range(8)), trace=trace)
    outs = [np.asarray(r["out"], dtype=np.float32).reshape(256, L) for r in res.results]
    out = np.empty((2, L, D_MODEL), np.float32)
    for b in range(2):
        fwd = outs[2 * b] + outs[2 * b + 1]
        bwd = outs[4 + 2 * b] + outs[4 + 2 * b + 1]
        out[b] = (fwd + bwd[:, ::-1]).T
    return out, res


def kernel(**inputs):
    out, _ = _run(inputs, trace=False)
    return out

